# revision 1
# baseline (speedup 1.0000x reference)
"""NT-Xent contrastive loss (B=4096, D=256, T=0.2) on 8 Trainium2 NeuronCores.

Strategy (data-parallel over rows of Z = concat([z_i, z_j])):
  Each core receives the full embedding matrix, rotated so that its own
  1024-row block sits at rows [0, 1024) (pure host-side layout op, zero
  FLOPs).  On-device, each core:
    1. loads the full [8192, 256] fp32 matrix, computes row norms
       (sum-of-squares via tensor_tensor_reduce, rsqrt via exp(-0.5*ln)),
    2. normalizes rows + casts to bf16, xbar-DMA-transposes to z^T
       ([256, 8192], the layout the PE needs),
    3. computes its 1024x8192 block of the similarity matrix with bf16
       matmuls (K=256 accumulated in PSUM fp32),
    4. ACT evaluates exp(5*sim) straight out of PSUM with accum_out
       producing the row sums (denominators) for free,
    5. subtracts exp(5) (the self-similarity term: unit rows => diag == 1),
       takes ln, combines with the positive-pair dot products, and reduces
       to a single partial-loss scalar.
  The host sums the 8 partial scalars (pure gather/reduce glue).
"""

import os
import sys

sys.path.insert(0, "/opt/trn_rl_repo")

_DEBUG = os.environ.get("KERNEL_DEBUG", "")
_TBOUNCE = os.environ.get("KERNEL_TBOUNCE", "") == "1"

import numpy as np

import concourse.bass as bass  # noqa: F401  (registers AP machinery)
import concourse.tile as tile
import concourse.mybir as mybir
from concourse import bacc, bass_utils

N_CORES = 8
B = 4096                 # rows per input matrix
R = 2 * B                # 8192 rows of Z
D = 256                  # embedding dim
BLK = R // N_CORES       # 1024 rows per core
P = 128                  # SBUF partitions
N_CHUNKS = R // P        # 64 row-chunks
GROUPS = 8               # norm/normalize/transpose pipeline groups
CHUNKS_PER_GROUP = N_CHUNKS // GROUPS
BLK_CHUNKS = BLK // P    # 8 chunks of own rows
PARTNER_OFF = B // P     # partner rows start 4096 rows (32 chunks) in
INV_T = 5.0              # 1 / temperature
EXP_DIAG = float(np.exp(5.0))  # exp(sim_rr / T) with unit-norm rows
NT = 512                 # moving free dim per matmul (one PSUM bank)
NSUB = 4                 # matmuls per PSUM tile (4 banks)
NBLKS = R // (NT * NSUB)  # 4 column blocks of 2048
M_CHUNKS = BLK // P      # 8 stationary chunks per core

F32 = mybir.dt.float32
BF16 = mybir.dt.bfloat16
AX = mybir.AxisListType
ALU = mybir.AluOpType
AF = mybir.ActivationFunctionType

_prog = None


def _patch_act_tables():
    """Make natural_log_exp_and_others the only provider of Exp/Ln so the
    table-load pass emits ONE load instead of thrashing between the
    exp-only and ln-only sets (measured 17 loads / 21.8us without this).
    Set ids are positional, so membership edits don't change ids."""
    if getattr(bacc, "_act_tables_patched", False):
        return
    orig = bacc.get_activation_tables

    def patched(arch):
        t = orig(arch)
        for name, funcs in t.items():
            if name != "natural_log_exp_and_others":
                funcs.discard(AF.Exp)
                funcs.discard(AF.Ln)
        return t

    bacc.get_activation_tables = patched
    bacc._act_tables_patched = True


def _build():
    _patch_act_tables()
    nc = bacc.Bacc(
        "TRN2", target_bir_lowering=False, debug=False, num_devices=N_CORES
    )
    x = nc.dram_tensor("x", [R, D], F32, kind="ExternalInput").ap()
    out = nc.dram_tensor("out", [1, 1], F32, kind="ExternalOutput").ap()

    ztmp = nc.dram_tensor("ztmp", [R, D], BF16, kind="Internal").ap()

    with tile.TileContext(nc) as tc:
        with tc.tile_pool(name="big", bufs=1) as big, \
             tc.tile_pool(name="small", bufs=1) as small, \
             tc.tile_pool(name="scratch", bufs=2) as scratch, \
             tc.tile_pool(name="esc", bufs=4) as esc, \
             tc.tile_pool(name="psum", bufs=2, space="PSUM") as psum:

            raw = big.tile([P, N_CHUNKS, D], F32)      # 64 KiB/part
            znat0 = big.tile([P, R], BF16)             # z rows, d in [0,128)
            znat1 = big.tile([P, R], BF16)             # z rows, d in [128,256)
            zt0 = big.tile([P, R], BF16)               # z^T rows 0..127 of D
            zt1 = big.tile([P, R], BF16)               # z^T rows 128..255 of D

            ss = small.tile([P, N_CHUNKS], F32)        # row mean-of-squares
            mv = small.tile([P, N_CHUNKS, 2], F32)     # bn_aggr mean/var
            msq = small.tile([P, N_CHUNKS], F32)
            inv = small.tile([P, N_CHUNKS], F32)       # 1/row-norm
            lnss = small.tile([P, N_CHUNKS], F32)
            dotraw = small.tile([P, BLK_CHUNKS], F32)  # raw pos dot products
            posb = small.tile([P, BLK_CHUNKS], F32)    # normalized positives
            acc = small.tile([P, M_CHUNKS, NBLKS], F32)  # exp row-sum partials
            rows = small.tile([P, M_CHUNKS], F32)      # denominators
            lnd = small.tile([P, M_CHUNKS], F32)       # ln(denominator)
            comb = small.tile([P, M_CHUNKS], F32)
            partial = small.tile([P, 1], F32)
            ones = small.tile([P, 1], F32)
            outsb = small.tile([1, 1], F32)

            nc.vector.memset(ones, 1.0 / float(R))
            nbias = small.tile([P, 1], F32)
            nc.vector.memset(nbias, -float(np.log(np.sqrt(D))))

            # ---- Phase A/B: load, norms, normalize+cast, transpose ----
            # grouped so the main loop can start before all rows are in
            for g in range(GROUPS):
                c0 = g * CHUNKS_PER_GROUP
                r0 = c0 * P
                r1 = r0 + CHUNKS_PER_GROUP * P
                nc.gpsimd.dma_start(
                    out=raw[:, c0:c0 + CHUNKS_PER_GROUP, :],
                    in_=x[r0:r1, :].rearrange("(c p) d -> p c d", p=P),
                )
                for ci in range(c0, c0 + CHUNKS_PER_GROUP):
                    # E[x^2] per row via bn_stats (tensor_tensor_reduce
                    # faults TRN2 here; bn_stats is the production path).
                    stats = scratch.tile([P, 6], F32)
                    nc.vector.bn_stats(out=stats, in_=raw[:, ci, :])
                    nc.vector.bn_aggr(out=mv[:, ci, :], in_=stats)
                gs = slice(c0, c0 + CHUNKS_PER_GROUP)
                # ss/D = var + mean^2
                nc.vector.tensor_mul(
                    msq[:, gs], mv[:, gs, 0], mv[:, gs, 0]
                )
                nc.vector.tensor_add(ss[:, gs], mv[:, gs, 1], msq[:, gs])
                # 1/norm = exp(-0.5*ln(ss/D) - ln(sqrt(D))); ACT Rsqrt is
                # banned; Ln/Exp share one table set with the main-loop Exp.
                nc.scalar.activation(lnss[:, gs], ss[:, gs], AF.Ln)
                nc.scalar.activation(
                    inv[:, gs], lnss[:, gs], AF.Exp,
                    scale=-0.5, bias=nbias,
                )
                for ci in range(c0, c0 + CHUNKS_PER_GROUP):
                    cs = slice(ci * P, (ci + 1) * P)
                    nc.vector.tensor_scalar_mul(
                        znat0[:, cs], raw[:, ci, 0:P], inv[:, ci:ci + 1]
                    )
                    nc.vector.tensor_scalar_mul(
                        znat1[:, cs], raw[:, ci, P:D], inv[:, ci:ci + 1]
                    )
                if _TBOUNCE:
                    # DRAM-bounce transpose (fallback; proven path)
                    nc.gpsimd.dma_start(
                        out=ztmp[r0:r1, 0:P].rearrange("(c p) d -> p c d", p=P),
                        in_=znat0[:, r0:r1].rearrange("p (c q) -> p c q", q=P),
                    )
                    nc.gpsimd.dma_start(
                        out=ztmp[r0:r1, P:D].rearrange("(c p) d -> p c d", p=P),
                        in_=znat1[:, r0:r1].rearrange("p (c q) -> p c q", q=P),
                    )
                    nc.sync.dma_start_transpose(
                        out=zt0[:, r0:r1], in_=ztmp[r0:r1, 0:P]
                    )
                    nc.sync.dma_start_transpose(
                        out=zt1[:, r0:r1], in_=ztmp[r0:r1, P:D]
                    )
                else:
                    # direct SBUF->SBUF xbar transpose, one per d-half
                    nc.sync.dma_start_transpose(
                        out=zt0[:, r0:r1].rearrange("q (a p) -> q a p", p=P),
                        in_=znat0[:, r0:r1],
                    )
                    nc.sync.dma_start_transpose(
                        out=zt1[:, r0:r1].rearrange("q (a p) -> q a p", p=P),
                        in_=znat1[:, r0:r1],
                    )

            # ---- positives: pos_r = (e_r . e_{r+B}) * inv_r * inv_{r+B} ----
            for j in range(BLK_CHUNKS):
                sqp = scratch.tile([P, D], F32)
                nc.vector.tensor_mul(
                    sqp, raw[:, j, :], raw[:, j + PARTNER_OFF, :]
                )
                nc.vector.tensor_reduce(
                    dotraw[:, j:j + 1], sqp, axis=AX.X, op=ALU.add
                )
            nc.vector.tensor_mul(posb, dotraw, inv[:, 0:BLK_CHUNKS])
            nc.vector.tensor_mul(
                posb, posb, inv[:, PARTNER_OFF:PARTNER_OFF + BLK_CHUNKS]
            )

            # ---- main loop: sim block matmuls + exp row-sums ----
            if _DEBUG == "nomain":
                nc.vector.memset(acc, 1000.0)
            for nb in range(NBLKS if _DEBUG != "nomain" else 0):
                for m in range(M_CHUNKS):
                    pt = psum.tile([P, NSUB * NT], F32)
                    for k, zt in enumerate((zt0, zt1)):
                        for ns in range(NSUB):
                            col = nb * (NSUB * NT) + ns * NT
                            nc.tensor.matmul(
                                pt[:, ns * NT:(ns + 1) * NT],
                                zt[:, m * P:(m + 1) * P],
                                zt[:, col:col + NT],
                                start=(k == 0),
                                stop=(k == 1),
                            )
                    ex = esc.tile([P, NSUB * NT], BF16)
                    nc.scalar.activation(
                        ex,
                        pt,
                        AF.Exp,
                        scale=INV_T,
                        accum_out=acc[:, m, nb:nb + 1],
                    )

            # ---- finalize ----
            for m in range(M_CHUNKS):
                nc.vector.tensor_reduce(
                    rows[:, m:m + 1], acc[:, m, :], axis=AX.X, op=ALU.add
                )
            nc.vector.tensor_scalar_add(rows, rows, -EXP_DIAG)
            nc.scalar.activation(lnd, rows, AF.Ln)
            # comb = ln(denom) - pos/T ; partial[p] = sum_m comb[p, m]
            nc.vector.scalar_tensor_tensor(
                out=comb,
                in0=posb,
                scalar=-INV_T,
                in1=lnd,
                op0=ALU.mult,
                op1=ALU.add,
                accum_out=partial,
            )
            fin = psum.tile([P, NSUB * NT], F32, tag="pt")
            nc.tensor.matmul(
                fin[0:1, 0:1], partial, ones, start=True, stop=True
            )
            nc.vector.tensor_copy(outsb, fin[0:1, 0:1])
            nc.sync.dma_start(out=out, in_=outsb)

    nc.compile()
    return nc


def _get_prog():
    global _prog
    if _prog is None:
        _prog = _build()
    return _prog


def kernel(emb_i: np.ndarray, emb_j: np.ndarray) -> np.ndarray:
    nc = _get_prog()
    z = np.concatenate(
        [np.asarray(emb_i, np.float32), np.asarray(emb_j, np.float32)], axis=0
    )
    in_maps = [
        {"x": np.ascontiguousarray(np.roll(z, -c * BLK, axis=0))}
        for c in range(N_CORES)
    ]
    res = bass_utils.run_bass_kernel_spmd(
        nc, in_maps, core_ids=list(range(N_CORES))
    )
    total = sum(float(res.results[c]["out"][0, 0]) for c in range(N_CORES))
    return np.asarray(total, dtype=np.float32)



# revision 2
# speedup vs baseline: 1.0817x; 1.0817x over previous
"""NT-Xent contrastive loss (B=4096, D=256, T=0.2) on 8 Trainium2 NeuronCores.

v2: fp8 DoubleRow matmuls + pipelined preamble.

Per core (data-parallel over rows of Z = concat([z_i, z_j]); host rotates
Z so each core's 1024-row block sits at rows [0, 1024)):
  1. DMA the full [8192, 256] fp32 matrix in 8 groups of 1024 rows.
  2. Per group, on DVE only (keeps ACT free for the main loop):
     ss = row sums of squares via tensor_scalar(pow 2, accum_out);
     inv8 = 8/||e|| via pow(-0.5) (fallback: quake rsqrt + Newton);
     znat8 = fp8e4(e * inv8)  (tensor_scalar 2x_2p).
  3. Pair-transpose znat8 (viewed as uint16) into ztp[p, c, i] =
     z^T[2p+i, c] -- the DoubleRow K=256-in-one-pass layout.
  4. Main loop over 4 column-gpairs x 8 m-chunks: one [128,2048] PSUM
     tile per (gp, m) filled by 4 DoubleRow matmuls (107 ns each), then
     ACT exp(psum * 5/64) with accum_out giving row sums.
  5. denom = sum - exp(5); loss partial via ln + positive-pair dots
     (raw fp32, exact); ones-matmul partition reduce; host sums 8 scalars.
"""

import os
import sys

sys.path.insert(0, "/opt/trn_rl_repo")

import numpy as np

import concourse.bass as bass  # noqa: F401  (registers AP machinery)
import concourse.tile as tile
import concourse.mybir as mybir
from concourse import bacc, bass_utils

N_CORES = 8
B = 4096                 # rows per input matrix
R = 2 * B                # 8192 rows of Z
D = 256                  # embedding dim
BLK = R // N_CORES       # 1024 rows per core
P = 128                  # SBUF partitions
N_CHUNKS = R // P        # 64 row-chunks
GROUPS = 8               # preamble pipeline groups
CPG = N_CHUNKS // GROUPS  # 8 chunks per group
PARTNER_OFF = B // P     # partner rows start 4096 rows (32 chunks) in
M_CHUNKS = BLK // P      # 8 stationary chunks per core
GPAIRS = 4               # column gpairs of 2048 in the main loop
NT = 512                 # matmul moving width (one PSUM bank)
S8 = 8.0                 # fp8 scale: zf8 = 8 * z / ||z||
EXP_SCALE = 5.0 / (S8 * S8)    # psum = 64*sim -> exp(psum * 5/64)
EXP_DIAG = float(np.exp(5.0))  # self-similarity term (unit rows)

F32 = mybir.dt.float32
BF16 = mybir.dt.bfloat16
FP8 = mybir.dt.float8e4
U16 = mybir.dt.uint16
AX = mybir.AxisListType
ALU = mybir.AluOpType
AF = mybir.ActivationFunctionType
PM = mybir.MatmulPerfMode

NORM_MODE = "quake"

# Schraudolph fast-exp constants for the DVE-offloaded tiles:
# i32 = psum*EXP_SCALE*2^23/ln2 + (127*2^23 - C); bitcast(i32) ~ exp(psum*EXP_SCALE)
# C calibrated for zero-mean relative error over the sim distribution.
SCH_A = EXP_SCALE * (2.0 ** 23) / float(np.log(2.0))
SCH_B = 127.0 * 2.0 ** 23 - 477742.0
# (gp, m) tiles whose exp runs on DVE instead of ACT (gp>=2: DVE has
# finished the normalize pipeline by then)
DVE_TILES = {(3, 1), (3, 3), (3, 5), (3, 7)}

_prog = None


def _patch_act_tables():
    """Make natural_log_exp_and_others the only provider of Exp/Ln so the
    table-load pass emits ONE load (ids are positional; membership edits
    don't change ids)."""
    if getattr(bacc, "_act_tables_patched", False):
        return
    orig = bacc.get_activation_tables

    def patched(arch):
        t = orig(arch)
        for name, funcs in t.items():
            if name != "natural_log_exp_and_others":
                funcs.discard(AF.Exp)
                funcs.discard(AF.Ln)
        return t

    bacc.get_activation_tables = patched
    bacc._act_tables_patched = True


def _build():
    _patch_act_tables()
    nc = bacc.Bacc(
        "TRN2", target_bir_lowering=False, debug=False, num_devices=N_CORES
    )
    x = nc.dram_tensor("x", [R, D], F32, kind="ExternalInput").ap()
    out = nc.dram_tensor("out", [1, 1], F32, kind="ExternalOutput").ap()

    with tile.TileContext(nc) as tc:
        with tc.tile_pool(name="big", bufs=1) as big, \
             tc.tile_pool(name="small", bufs=1) as small, \
             tc.tile_pool(name="sq", bufs=4) as sqp, \
             tc.tile_pool(name="esc", bufs=2) as esc, \
             tc.tile_pool(name="sch", bufs=2) as sch, \
             tc.tile_pool(name="psum", bufs=2, space="PSUM") as psum:

            raw = big.tile([P, N_CHUNKS, D], F32)      # 64 KiB/part
            znat8 = big.tile([P, N_CHUNKS, D], FP8)    # 16 KiB/part
            ztp = big.tile([P, R, 2], FP8)             # 16 KiB/part
            ztm = big.tile([P, 2, BLK], FP8)           # k-major stationary

            znat16 = znat8.bitcast(U16)                # [P, N_CHUNKS, 128]
            ztp16 = ztp.bitcast(U16)                   # [P, R]

            ss = small.tile([P, N_CHUNKS], F32)        # row sums of squares
            inv8 = small.tile([P, N_CHUNKS], F32)      # 8/row-norm
            dotraw = small.tile([P, M_CHUNKS], F32)    # raw pos dot products
            posb = small.tile([P, M_CHUNKS], F32)
            acc = small.tile([P, M_CHUNKS, GPAIRS], F32)
            rows = small.tile([P, M_CHUNKS], F32)      # denominators
            lnd = small.tile([P, M_CHUNKS], F32)
            comb = small.tile([P, M_CHUNKS], F32)
            partial = small.tile([P, 1], F32)
            ones = small.tile([P, 1], F32)
            outsb = small.tile([1, 1], F32)

            nc.vector.memset(ones, 1.0 / float(R))

            if NORM_MODE == "quake":
                q_i32 = small.tile([P, N_CHUNKS], mybir.dt.int32)
                q_f32 = q_i32.bitcast(F32)
                nwt = small.tile([P, N_CHUNKS], F32)

            # ---- preamble: load, norms, inv8, normalize, pair-transpose ----
            # all input loads issued up-front on the SP HWDGE queue
            for g in range(GROUPS):
                c0 = g * CPG
                r0 = c0 * P
                nc.sync.dma_start(
                    out=raw[:, c0:c0 + CPG, :],
                    in_=x[r0:r0 + CPG * P, :].rearrange("(c p) d -> p c d", p=P),
                )
            for g in range(GROUPS):
                c0 = g * CPG
                gs = slice(c0, c0 + CPG)
                r0 = c0 * P
                for ci in range(c0, c0 + CPG):
                    sqt = sqp.tile([P, D], BF16)
                    nc.vector.scalar_tensor_tensor(
                        out=sqt, in0=raw[:, ci, :], scalar=1.0,
                        in1=raw[:, ci, :],
                        op0=ALU.mult, op1=ALU.mult,
                        accum_out=ss[:, ci:ci + 1],
                    )
                if NORM_MODE == "pow":
                    # inv8 = 8 * ss^-0.5 in one DVE pass
                    nc.vector.tensor_scalar(
                        out=inv8[:, gs], in0=ss[:, gs],
                        scalar1=-0.5, scalar2=S8, op0=ALU.pow, op1=ALU.mult,
                    )
                else:
                    # quake rsqrt seed + 2 Newton iterations, all DVE
                    ss_i32 = ss.bitcast(mybir.dt.int32)
                    # seed = 0x5F3759DF - (i >> 1); bitwise and arith ops
                    # cannot mix in one tensor_scalar.
                    nc.vector.tensor_scalar(
                        out=q_i32[:, gs], in0=ss_i32[:, gs],
                        scalar1=1, scalar2=None,
                        op0=ALU.logical_shift_right,
                    )
                    nc.vector.tensor_scalar(
                        out=q_i32[:, gs], in0=q_i32[:, gs],
                        scalar1=0x5F3759DF, scalar2=-1,
                        op0=ALU.subtract, op1=ALU.mult,
                    )
                    for _ in range(1):
                        # y <- y * (1.5 - 0.5*ss*y^2)
                        nc.vector.tensor_tensor(
                            out=nwt[:, gs], in0=q_f32[:, gs], in1=q_f32[:, gs],
                            op=ALU.mult,
                        )
                        nc.vector.tensor_tensor(
                            out=nwt[:, gs], in0=nwt[:, gs], in1=ss[:, gs],
                            op=ALU.mult,
                        )
                        nc.vector.tensor_scalar(
                            out=nwt[:, gs], in0=nwt[:, gs],
                            scalar1=-0.5, scalar2=1.5, op0=ALU.mult, op1=ALU.add,
                        )
                        nc.vector.tensor_tensor(
                            out=q_f32[:, gs], in0=q_f32[:, gs], in1=nwt[:, gs],
                            op=ALU.mult,
                        )
                    nc.vector.tensor_scalar(
                        out=inv8[:, gs], in0=q_f32[:, gs],
                        scalar1=S8, scalar2=None, op0=ALU.mult,
                    )
                for ci in range(c0, c0 + CPG):
                    nc.vector.tensor_scalar_mul(
                        znat8[:, ci, :], raw[:, ci, :], inv8[:, ci:ci + 1]
                    )
                nc.sync.dma_start_transpose(
                    out=ztp16[:, r0:r0 + CPG * P, :].rearrange(
                        "q (a p) o -> q a (p o)", p=P
                    ),
                    in_=znat16[:, gs, :],
                )
                if g == 0:
                    # k-major copy of own rows for ldweights (the pair-
                    # interleaved layout violates s3_lw_dual_fp8 rules)
                    for i in range(2):
                        nc.vector.tensor_copy(
                            ztm[:, i, :], ztp[:, 0:BLK, i]
                        )

            # ---- positives: pos_r = (e_r . e_{r+B}) raw fp32 ----
            for j in range(M_CHUNKS):
                pscr = sqp.tile([P, D], BF16)
                nc.vector.scalar_tensor_tensor(
                    out=pscr, in0=raw[:, j, :], scalar=1.0,
                    in1=raw[:, j + PARTNER_OFF, :],
                    op0=ALU.mult, op1=ALU.mult,
                    accum_out=dotraw[:, j:j + 1],
                )
            nc.vector.tensor_mul(posb, dotraw, inv8[:, 0:M_CHUNKS])
            nc.vector.tensor_mul(
                posb, posb, inv8[:, PARTNER_OFF:PARTNER_OFF + M_CHUNKS]
            )

            # ---- main loop: DoubleRow matmuls + exp row-sums ----
            for gp in range(GPAIRS):
                for m in range(M_CHUNKS):
                    pt = psum.tile([P, 4 * NT], F32)
                    lhsT = ztm[:, :, m * P:(m + 1) * P]
                    for b in range(4):
                        col = gp * (4 * NT) + b * NT
                        nc.tensor.matmul(
                            pt[:, b * NT:(b + 1) * NT],
                            lhsT,
                            ztp[:, col:col + NT, :].rearrange("p c i -> p i c"),
                            start=True,
                            stop=True,
                            perf_mode=PM.DoubleRow,
                        )
                    if (gp, m) in DVE_TILES:
                        # Schraudolph fast exp on DVE: affine into int32,
                        # bitcast back as fp32 ~ exp, then sum.
                        q32 = sch.tile([P, 4 * NT], mybir.dt.int32)
                        nc.vector.tensor_scalar(
                            out=q32, in0=pt,
                            scalar1=SCH_A, scalar2=SCH_B,
                            op0=ALU.mult, op1=ALU.add,
                        )
                        exf = esc.tile([P, 4 * NT], BF16)
                        nc.vector.tensor_scalar(
                            out=exf, in0=q32.bitcast(F32),
                            scalar1=1.0, scalar2=None,
                            op0=ALU.mult, op1=ALU.add,
                            accum_out=acc[:, m, gp:gp + 1],
                        )
                    else:
                        ex = esc.tile([P, 4 * NT], BF16)
                        nc.scalar.activation(
                            ex,
                            pt,
                            AF.Exp,
                            scale=EXP_SCALE,
                            accum_out=acc[:, m, gp:gp + 1],
                        )

            # ---- finalize ----
            for m in range(M_CHUNKS):
                nc.vector.tensor_reduce(
                    rows[:, m:m + 1], acc[:, m, :], axis=AX.X, op=ALU.add
                )
            nc.vector.tensor_scalar_add(rows, rows, -EXP_DIAG)
            nc.scalar.activation(lnd, rows, AF.Ln)
            # comb = ln(denom) - 5*pos ; pos = dotraw*inv8_r*inv8_p/64
            nc.vector.scalar_tensor_tensor(
                out=comb,
                in0=posb,
                scalar=-5.0 / (S8 * S8),
                in1=lnd,
                op0=ALU.mult,
                op1=ALU.add,
                accum_out=partial,
            )
            fin = psum.tile([P, 4 * NT], F32, tag="pt")
            nc.tensor.matmul(
                fin[0:1, 0:1], partial, ones, start=True, stop=True
            )
            nc.vector.tensor_copy(outsb, fin[0:1, 0:1])
            nc.sync.dma_start(out=out, in_=outsb)

    nc.compile()
    return nc


def _get_prog():
    global _prog
    if _prog is None:
        _prog = _build()
    return _prog


def kernel(emb_i: np.ndarray, emb_j: np.ndarray) -> np.ndarray:
    nc = _get_prog()
    z = np.concatenate(
        [np.asarray(emb_i, np.float32), np.asarray(emb_j, np.float32)], axis=0
    )
    in_maps = [
        {"x": np.ascontiguousarray(np.roll(z, -c * BLK, axis=0))}
        for c in range(N_CORES)
    ]
    res = bass_utils.run_bass_kernel_spmd(
        nc, in_maps, core_ids=list(range(N_CORES))
    )
    total = sum(float(res.results[c]["out"][0, 0]) for c in range(N_CORES))
    return np.asarray(total, dtype=np.float32)


# revision 3
# speedup vs baseline: 1.1378x; 1.0519x over previous
"""NT-Xent contrastive loss (B=4096, D=256, T=0.2) on 8 Trainium2 NeuronCores.

v2: fp8 DoubleRow matmuls + pipelined preamble.

Per core (data-parallel over rows of Z = concat([z_i, z_j]); host rotates
Z so each core's 1024-row block sits at rows [0, 1024)):
  1. DMA the full [8192, 256] fp32 matrix in 8 groups of 1024 rows.
  2. Per group, on DVE only (keeps ACT free for the main loop):
     ss = row sums of squares via tensor_scalar(pow 2, accum_out);
     inv8 = 8/||e|| via pow(-0.5) (fallback: quake rsqrt + Newton);
     znat8 = fp8e4(e * inv8)  (tensor_scalar 2x_2p).
  3. Pair-transpose znat8 (viewed as uint16) into ztp[p, c, i] =
     z^T[2p+i, c] -- the DoubleRow K=256-in-one-pass layout.
  4. Main loop over 4 column-gpairs x 8 m-chunks: one [128,2048] PSUM
     tile per (gp, m) filled by 4 DoubleRow matmuls (107 ns each), then
     ACT exp(psum * 5/64) with accum_out giving row sums.
  5. denom = sum - exp(5); loss partial via ln + positive-pair dots
     (raw fp32, exact); ones-matmul partition reduce; host sums 8 scalars.
"""

import os
import sys

sys.path.insert(0, "/opt/trn_rl_repo")

import numpy as np

import concourse.bass as bass  # noqa: F401  (registers AP machinery)
import concourse.tile as tile
import concourse.mybir as mybir
from concourse import bacc, bass_utils

N_CORES = 8
B = 4096                 # rows per input matrix
R = 2 * B                # 8192 rows of Z
D = 256                  # embedding dim
BLK = R // N_CORES       # 1024 rows per core
P = 128                  # SBUF partitions
N_CHUNKS = R // P        # 64 row-chunks
GROUPS = 8               # preamble pipeline groups
CPG = N_CHUNKS // GROUPS  # 8 chunks per group
PARTNER_OFF = B // P     # partner rows start 4096 rows (32 chunks) in
M_CHUNKS = BLK // P      # 8 stationary chunks per core
GPAIRS = 4               # column gpairs of 2048 in the main loop
NT = 512                 # matmul moving width (one PSUM bank)
S8 = 8.0                 # fp8 scale: zf8 = 8 * z / ||z||
EXP_SCALE = 5.0 / (S8 * S8)    # psum = 64*sim -> exp(psum * 5/64)
EXP_DIAG = float(np.exp(5.0))  # self-similarity term (unit rows)

F32 = mybir.dt.float32
BF16 = mybir.dt.bfloat16
FP8 = mybir.dt.float8e4
U16 = mybir.dt.uint16
AX = mybir.AxisListType
ALU = mybir.AluOpType
AF = mybir.ActivationFunctionType
PM = mybir.MatmulPerfMode

NORM_MODE = "quake"

# Schraudolph fast-exp constants for the DVE-offloaded tiles:
# i32 = psum*EXP_SCALE*2^23/ln2 + (127*2^23 - C); bitcast(i32) ~ exp(psum*EXP_SCALE)
# C calibrated for zero-mean relative error over the sim distribution.
SCH_A = EXP_SCALE * (2.0 ** 7) / float(np.log(2.0))
SCH_B = 127.0 * 2.0 ** 7 - 477742.0 / 65536.0
# (gp, m) tiles whose exp runs on DVE instead of ACT (gp>=2: DVE has
# finished the normalize pipeline by then)
DVE_TILES = {(3, 1), (3, 3), (3, 5), (3, 7)}

_prog = None


def _patch_act_tables():
    """Make natural_log_exp_and_others the only provider of Exp/Ln so the
    table-load pass emits ONE load (ids are positional; membership edits
    don't change ids)."""
    if getattr(bacc, "_act_tables_patched", False):
        return
    orig = bacc.get_activation_tables

    def patched(arch):
        t = orig(arch)
        for name, funcs in t.items():
            if name != "natural_log_exp_and_others":
                funcs.discard(AF.Exp)
                funcs.discard(AF.Ln)
        return t

    bacc.get_activation_tables = patched
    bacc._act_tables_patched = True


def _build():
    _patch_act_tables()
    nc = bacc.Bacc(
        "TRN2", target_bir_lowering=False, debug=False, num_devices=N_CORES
    )
    x = nc.dram_tensor("x", [R, D], F32, kind="ExternalInput").ap()
    out = nc.dram_tensor("out", [1, 1], F32, kind="ExternalOutput").ap()

    with tile.TileContext(nc) as tc:
        with tc.tile_pool(name="big", bufs=1) as big, \
             tc.tile_pool(name="small", bufs=1) as small, \
             tc.tile_pool(name="sq", bufs=4) as sqp, \
             tc.tile_pool(name="esc", bufs=2) as esc, \
             tc.tile_pool(name="sch", bufs=2) as sch, \
             tc.tile_pool(name="psum", bufs=2, space="PSUM") as psum:

            raw = big.tile([P, N_CHUNKS, D], F32)      # 64 KiB/part
            znat8 = big.tile([P, N_CHUNKS, D], FP8)    # 16 KiB/part
            ztp = big.tile([P, R, 2], FP8)             # 16 KiB/part
            ztm = big.tile([P, 2, BLK], FP8)           # k-major stationary

            znat16 = znat8.bitcast(U16)                # [P, N_CHUNKS, 128]
            ztp16 = ztp.bitcast(U16)                   # [P, R]

            ss = small.tile([P, N_CHUNKS], F32)        # row sums of squares
            inv8 = small.tile([P, N_CHUNKS], F32)      # 8/row-norm
            dotraw = small.tile([P, M_CHUNKS], F32)    # raw pos dot products
            posb = small.tile([P, M_CHUNKS], F32)
            acc = small.tile([P, M_CHUNKS, GPAIRS], F32)
            rows = small.tile([P, M_CHUNKS], F32)      # denominators
            lnd = small.tile([P, M_CHUNKS], F32)
            comb = small.tile([P, M_CHUNKS], F32)
            partial = small.tile([P, 1], F32)
            ones = small.tile([P, 1], F32)
            outsb = small.tile([1, 1], F32)

            nc.vector.memset(ones, 1.0 / float(R))

            if NORM_MODE == "quake":
                q_i32 = small.tile([P, N_CHUNKS], mybir.dt.int32)
                q_f32 = q_i32.bitcast(F32)
                nwt = small.tile([P, N_CHUNKS], F32)

            # ---- preamble: load, norms, inv8, normalize, pair-transpose ----
            # all input loads issued up-front on the SP HWDGE queue
            for g in range(GROUPS):
                c0 = g * CPG
                r0 = c0 * P
                nc.sync.dma_start(
                    out=raw[:, c0:c0 + CPG, :],
                    in_=x[r0:r0 + CPG * P, :].rearrange("(c p) d -> p c d", p=P),
                )
            for g in range(GROUPS):
                c0 = g * CPG
                gs = slice(c0, c0 + CPG)
                r0 = c0 * P
                for ci in range(c0, c0 + CPG):
                    sqt = sqp.tile([P, D], BF16)
                    nc.vector.scalar_tensor_tensor(
                        out=sqt, in0=raw[:, ci, :], scalar=1.0,
                        in1=raw[:, ci, :],
                        op0=ALU.mult, op1=ALU.mult,
                        accum_out=ss[:, ci:ci + 1],
                    )
                if NORM_MODE == "pow":
                    # inv8 = 8 * ss^-0.5 in one DVE pass
                    nc.vector.tensor_scalar(
                        out=inv8[:, gs], in0=ss[:, gs],
                        scalar1=-0.5, scalar2=S8, op0=ALU.pow, op1=ALU.mult,
                    )
                else:
                    # quake rsqrt seed + 2 Newton iterations, all DVE
                    ss_i32 = ss.bitcast(mybir.dt.int32)
                    # seed = 0x5F3759DF - (i >> 1); bitwise and arith ops
                    # cannot mix in one tensor_scalar.
                    nc.vector.tensor_scalar(
                        out=q_i32[:, gs], in0=ss_i32[:, gs],
                        scalar1=1, scalar2=None,
                        op0=ALU.logical_shift_right,
                    )
                    nc.vector.tensor_scalar(
                        out=q_i32[:, gs], in0=q_i32[:, gs],
                        scalar1=0x5F3759DF, scalar2=-1,
                        op0=ALU.subtract, op1=ALU.mult,
                    )
                    for _ in range(1):
                        # y <- y * (1.5 - 0.5*ss*y^2)
                        nc.vector.tensor_tensor(
                            out=nwt[:, gs], in0=q_f32[:, gs], in1=q_f32[:, gs],
                            op=ALU.mult,
                        )
                        nc.vector.tensor_tensor(
                            out=nwt[:, gs], in0=nwt[:, gs], in1=ss[:, gs],
                            op=ALU.mult,
                        )
                        nc.vector.tensor_scalar(
                            out=nwt[:, gs], in0=nwt[:, gs],
                            scalar1=-0.5, scalar2=1.5, op0=ALU.mult, op1=ALU.add,
                        )
                        nc.vector.tensor_tensor(
                            out=q_f32[:, gs], in0=q_f32[:, gs], in1=nwt[:, gs],
                            op=ALU.mult,
                        )
                    nc.vector.tensor_scalar(
                        out=inv8[:, gs], in0=q_f32[:, gs],
                        scalar1=S8, scalar2=None, op0=ALU.mult,
                    )
                for ci in range(c0, c0 + CPG):
                    nc.vector.tensor_scalar_mul(
                        znat8[:, ci, :], raw[:, ci, :], inv8[:, ci:ci + 1]
                    )
                nc.sync.dma_start_transpose(
                    out=ztp16[:, r0:r0 + CPG * P, :].rearrange(
                        "q (a p) o -> q a (p o)", p=P
                    ),
                    in_=znat16[:, gs, :],
                )
                if g == 0:
                    # k-major copy of own rows for ldweights (the pair-
                    # interleaved layout violates s3_lw_dual_fp8 rules)
                    for i in range(2):
                        nc.vector.tensor_copy(
                            ztm[:, i, :], ztp[:, 0:BLK, i]
                        )

            # ---- positives: pos_r = (e_r . e_{r+B}) raw fp32 ----
            for j in range(M_CHUNKS):
                pscr = sqp.tile([P, D], BF16)
                nc.vector.scalar_tensor_tensor(
                    out=pscr, in0=raw[:, j, :], scalar=1.0,
                    in1=raw[:, j + PARTNER_OFF, :],
                    op0=ALU.mult, op1=ALU.mult,
                    accum_out=dotraw[:, j:j + 1],
                )
            nc.vector.tensor_mul(posb, dotraw, inv8[:, 0:M_CHUNKS])
            nc.vector.tensor_mul(
                posb, posb, inv8[:, PARTNER_OFF:PARTNER_OFF + M_CHUNKS]
            )

            # ---- main loop: DoubleRow matmuls + exp row-sums ----
            for gp in range(GPAIRS):
                for m in range(M_CHUNKS):
                    pt = psum.tile([P, 4 * NT], F32)
                    lhsT = ztm[:, :, m * P:(m + 1) * P]
                    for b in range(4):
                        col = gp * (4 * NT) + b * NT
                        nc.tensor.matmul(
                            pt[:, b * NT:(b + 1) * NT],
                            lhsT,
                            ztp[:, col:col + NT, :].rearrange("p c i -> p i c"),
                            start=True,
                            stop=True,
                            perf_mode=PM.DoubleRow,
                        )
                    if (gp, m) in DVE_TILES:
                        # Schraudolph fast exp on DVE: affine into int32,
                        # bitcast back as fp32 ~ exp, then sum.
                        q16 = sch.tile([P, 4 * NT], mybir.dt.int16)
                        nc.vector.tensor_scalar(
                            out=q16, in0=pt,
                            scalar1=SCH_A, scalar2=SCH_B,
                            op0=ALU.mult, op1=ALU.add,
                        )
                        exf = esc.tile([P, 4 * NT], BF16)
                        nc.vector.tensor_scalar(
                            out=exf, in0=q16.bitcast(BF16),
                            scalar1=1.0, scalar2=None,
                            op0=ALU.mult, op1=ALU.add,
                            accum_out=acc[:, m, gp:gp + 1],
                        )
                    else:
                        ex = esc.tile([P, 4 * NT], BF16)
                        nc.scalar.activation(
                            ex,
                            pt,
                            AF.Exp,
                            scale=EXP_SCALE,
                            accum_out=acc[:, m, gp:gp + 1],
                        )

            # ---- finalize ----
            for m in range(M_CHUNKS):
                nc.vector.tensor_reduce(
                    rows[:, m:m + 1], acc[:, m, :], axis=AX.X, op=ALU.add
                )
            nc.vector.tensor_scalar_add(rows, rows, -EXP_DIAG)
            nc.scalar.activation(lnd, rows, AF.Ln)
            # comb = ln(denom) - 5*pos ; pos = dotraw*inv8_r*inv8_p/64
            nc.vector.scalar_tensor_tensor(
                out=comb,
                in0=posb,
                scalar=-5.0 / (S8 * S8),
                in1=lnd,
                op0=ALU.mult,
                op1=ALU.add,
                accum_out=partial,
            )
            fin = psum.tile([P, 4 * NT], F32, tag="pt")
            nc.tensor.matmul(
                fin[0:1, 0:1], partial, ones, start=True, stop=True
            )
            nc.vector.tensor_copy(outsb, fin[0:1, 0:1])
            nc.sync.dma_start(out=out, in_=outsb)

    nc.compile()
    return nc


def _get_prog():
    global _prog
    if _prog is None:
        _prog = _build()
    return _prog


def kernel(emb_i: np.ndarray, emb_j: np.ndarray) -> np.ndarray:
    nc = _get_prog()
    z = np.concatenate(
        [np.asarray(emb_i, np.float32), np.asarray(emb_j, np.float32)], axis=0
    )
    in_maps = [
        {"x": np.ascontiguousarray(np.roll(z, -c * BLK, axis=0))}
        for c in range(N_CORES)
    ]
    res = bass_utils.run_bass_kernel_spmd(
        nc, in_maps, core_ids=list(range(N_CORES))
    )
    total = sum(float(res.results[c]["out"][0, 0]) for c in range(N_CORES))
    return np.asarray(total, dtype=np.float32)
